# revision 1
# baseline (speedup 1.0000x reference)
"""Multihead attention (B=2, S=2048, E=1024, H=16) on 8 TRN2 cores.

Sharding: tensor-parallel over heads — core c computes heads {2c, 2c+1}
(dout = 128 columns of the QKV projections) for the full sequence, then its
partial contribution to the output projection; the host sums the 8 partials
and adds the output bias.

Device layout (per core):
  activations are pre-transposed on host to x^T [E, B*S] (and rounded to
  bf16 — the bf16 matmuls would round them anyway) so the projection
  matmuls contract E on the partition dim.  QKV projections produce
  Q^T/K^T/V^T [128, 4096] in SBUF (bf16).  Attention per (batch, head)
  computes scores^T [kpos, q] tiles directly (lhsT = K^T slice,
  rhs = Q^T slice), exponentiates on the scalar engine (fp32 psum in,
  bf16 out), and multiplies by V via matmul with lhsT = [V | ones] so the
  softmax denominator falls out of the same accumulation (row 64 of the
  PSUM result).  context^T is normalized with a reciprocal + PE-replicated
  row (kept float32r), and the output projection (float32r = full fp32
  bits) contracts the 128 local head dims.

Emission order interleaves batch-0 attention with batch-1 projections and
batch-1 attention with batch-0 output projection so DMA streaming, PE,
ACT (exp) and DVE stay overlapped across the whole kernel.
"""

import numpy as np
import ml_dtypes

# Problem constants (hardcoded per the task contract).
B, S, E, H = 2, 2048, 1024, 16
D = E // H          # 64
NSEQ = B * S        # 4096
NCORES = 8
DOUT = E // NCORES  # 128 = 2 heads x 64
KE = E // 128       # 8 contraction tiles over E
SEQT = 512          # seq tile for projections / q-block for attention
NST = NSEQ // SEQT  # 8
QB = S // SEQT      # 4 q-blocks per batch
KT = S // 128       # 16 kpos tiles per batch
ISD = float(D) ** -0.5

_PROGRAM = None


# ---------------------------------------------------------------------------
# Workarounds for this walrus build: at most ONE sync wait per instruction is
# reliably accepted ("Too many sync wait commands").  (1) tile's final drain
# gets one wait per logical proc — split them over single-wait SP NOPs;
# (2) a general post-pass moves any instruction's excess waits onto
# preceding same-engine NOPs (engine program order preserves semantics).
# ---------------------------------------------------------------------------


def _install_tile_drain_patch():
    import concourse.mybir as mybir
    import concourse.tile as tile
    from concourse.tile import ScopedClock

    if getattr(tile.TileContext, "_drain_patch_installed", False):
        return

    def _patched_drain_and_barrier(self, tick_clock, wait_clock):
        nc = self.nc
        carrier = nc.sync.nop(nofuse=True)
        wait_clock.add_sem_waits(
            carrier.ins, ScopedClock({None: tick_clock.global_clock})
        )
        si = carrier.ins.sync_info
        waits = list(si.on_wait) if si and si.on_wait else []
        ups = list(si.on_update) if si and si.on_update else []
        if len(waits) > 1:
            carrier.ins.sync_info = mybir.SyncInfo(on_wait=[waits[0]], on_update=ups)
            for w in waits[1:]:
                n2 = nc.sync.nop(nofuse=True)
                n2.ins.sync_info = mybir.SyncInfo(on_wait=[w], on_update=[])
        nc.sync.drain()
        nc.all_engine_barrier()
        popped = nc._tile_sem_poison_stack.pop()
        assert popped is self._sem_poison
        nc.clear_and_free_semaphores(list(self.sems.allocated().values()))
        nc.all_engine_barrier()

    tile.TileContext._drain_and_barrier = _patched_drain_and_barrier
    tile.TileContext._drain_patch_installed = True


MAX_WAITS = 1


def _split_excess_waits(nc):
    import concourse.mybir as mybir

    for bb in nc.main_func.blocks:
        il = list(bb.instructions)
        out = []
        changed = False
        for ins in il:
            si = ins.sync_info
            waits = list(si.on_wait) if si and si.on_wait else []
            if len(waits) > MAX_WAITS:
                changed = True
                extras = waits[: len(waits) - MAX_WAITS]
                keep = waits[len(extras):]
                for i in range(0, len(extras), MAX_WAITS):
                    chunk = extras[i : i + MAX_WAITS]
                    nop = mybir.InstNoOp(
                        name=nc.get_next_instruction_name(), ins=[], outs=[]
                    )
                    nop.engine = ins.engine
                    nop.sync_info = mybir.SyncInfo(on_wait=chunk, on_update=[])
                    out.append(nop)
                ins.sync_info = mybir.SyncInfo(
                    on_wait=keep, on_update=list(si.on_update) if si.on_update else []
                )
            out.append(ins)
        if changed:
            bb.instructions = out


def _build_program():
    import concourse.bass as bass
    import concourse.mybir as mybir
    import concourse.tile as tile
    from concourse.masks import make_identity

    _install_tile_drain_patch()

    f32 = mybir.dt.float32
    f32r = mybir.dt.float32r
    bf16 = mybir.dt.bfloat16

    nc = bass.Bass("TRN2", target_bir_lowering=False, debug=False)

    # DRAM I/O (per core).  Activations/projection weights are bf16.
    xq = nc.dram_tensor("xq", [KE, 128, NSEQ], bf16, kind="ExternalInput").ap()
    xk = nc.dram_tensor("xk", [KE, 128, NSEQ], bf16, kind="ExternalInput").ap()
    xv = nc.dram_tensor("xv", [KE, 128, NSEQ], bf16, kind="ExternalInput").ap()
    wq = nc.dram_tensor("wq", [KE, 128, DOUT], bf16, kind="ExternalInput").ap()
    wk = nc.dram_tensor("wk", [KE, 128, DOUT], bf16, kind="ExternalInput").ap()
    wv = nc.dram_tensor("wv", [KE, 128, DOUT], bf16, kind="ExternalInput").ap()
    wo = nc.dram_tensor("wo", [DOUT, E], f32r, kind="ExternalInput").ap()
    bq = nc.dram_tensor("bq", [DOUT, 1], f32, kind="ExternalInput").ap()
    bk = nc.dram_tensor("bk", [DOUT, 1], f32, kind="ExternalInput").ap()
    bv = nc.dram_tensor("bv", [DOUT, 1], f32, kind="ExternalInput").ap()
    out = nc.dram_tensor("out", [NSEQ, E], f32, kind="ExternalOutput").ap()

    with tile.TileContext(nc) as tc:
        with (
            nc.allow_low_precision(reason="bf16/f32r attention pipeline"),
            tc.tile_pool(name="consts", bufs=1) as consts,
            tc.tile_pool(name="persist", bufs=1) as persist,
            tc.tile_pool(name="xstream", bufs=12) as xstream,
            tc.tile_pool(name="ptp", bufs=8) as ptp,
            tc.tile_pool(name="outp", bufs=4) as outp,
            tc.tile_pool(name="small", bufs=4) as small,
            tc.tile_pool(name="pp_ps", bufs=2, space="PSUM") as pp_ps,
            tc.tile_pool(name="sc_ps", bufs=4, space="PSUM") as sc_ps,
            tc.tile_pool(name="cx_ps", bufs=2, space="PSUM") as cx_ps,
        ):
            # ---- constants / persistent SBUF state ----
            ident_f32 = consts.tile([128, 128], f32)
            make_identity(nc, ident_f32[:])
            ident = consts.tile([128, 128], bf16)
            nc.vector.tensor_copy(ident[:], ident_f32[:])
            onesf = consts.tile([128, 1], f32)
            nc.vector.memset(onesf[:], 1.0)
            ones64 = consts.tile([1, 64], f32r)
            nc.vector.tensor_copy(ones64[:], onesf[0:1, 0:1].broadcast_to([1, 64]))

            w_sb = {}
            b_sb = {}
            for name, wdram, bdram in (("q", wq, bq), ("k", wk, bk), ("v", wv, bv)):
                wt = persist.tile([128, KE, DOUT], bf16, tag=f"w{name}")
                for k in range(KE):
                    nc.sync.dma_start(wt[:, k, :], wdram[k])
                w_sb[name] = wt
                bt = persist.tile([DOUT, 1], f32, tag=f"b{name}")
                nc.sync.dma_start(bt[:], bdram[:])
                b_sb[name] = bt
            wo_sb = persist.tile([DOUT, E], f32r, tag="wo")
            nc.sync.dma_start(wo_sb[:], wo[:])

            qt_sb = persist.tile([128, NSEQ], bf16, tag="qt")
            kt_sb = persist.tile([128, NSEQ], bf16, tag="kt")
            vt_sb = persist.tile([128, NSEQ], bf16, tag="vt")
            xT_sb = {"q": qt_sb, "k": kt_sb, "v": vt_sb}
            # [V | ones] per (kpos chunk, head): [128, 32, 2, 65] bf16
            v_sb = persist.tile([128, NSEQ // 128, 2, D + 1], bf16, tag="vn")
            nc.vector.tensor_copy(
                v_sb[:, :, :, D], onesf[:, 0:1].broadcast_to([128, NSEQ // 128, 2])
            )
            ctxT_sb = persist.tile([128, NSEQ], f32r, tag="ctxT")

            xdram = {"q": xq, "k": xk, "v": xv}

            def proj_step(st):
                sl = bass.ts(st, SEQT)
                for name in ("q", "k", "v"):
                    ps = pp_ps.tile([128, SEQT], f32, tag="pp", name=f"pp{st}{name}")
                    for k in range(KE):
                        xt = xstream.tile([128, SEQT], bf16, tag="xs", name="xt")
                        nc.sync.dma_start(xt[:], xdram[name][k, :, sl])
                        nc.tensor.matmul(
                            ps[:],
                            lhsT=w_sb[name][:, k, :],
                            rhs=xt[:],
                            start=(k == 0),
                            stop=(k == KE - 1),
                        )
                    nc.vector.tensor_scalar_add(
                        xT_sb[name][:, sl], ps[:], b_sb[name][:, 0:1]
                    )
                # transpose this slice of V^T into [V | ones] chunks
                for ci in range(st * (SEQT // 128), (st + 1) * (SEQT // 128)):
                    tp = pp_ps.tile([128, 128], bf16, tag="pp", name="tp")
                    nc.tensor.transpose(
                        tp[:], vt_sb[:, bass.ts(ci, 128)], ident[:]
                    )
                    for h in range(2):
                        nc.vector.tensor_copy(
                            v_sb[:, ci, h, 0:D], tp[:, bass.ts(h, D)]
                        )

            def attn_step(b, qb):
                qsl = bass.ds(b * S + qb * SEQT, SEQT)
                ctx = [None, None]
                for h in range(2):
                    ctx[h] = cx_ps.tile([D + 1, SEQT], f32, tag="cx", name=f"ctx{h}")
                for t in range(KT):
                    ksl = bass.ds(b * S + t * 128, 128)
                    pt = [None, None]
                    for h in range(2):
                        hsl = bass.ts(h, D)
                        sc = sc_ps.tile([128, SEQT], f32, tag="sc", name=f"sc{h}")
                        nc.tensor.matmul(
                            sc[:],
                            lhsT=kt_sb[hsl, ksl],
                            rhs=qt_sb[hsl, qsl],
                            start=True,
                            stop=True,
                        )
                        pt[h] = ptp.tile([128, SEQT], bf16, tag="pt", name=f"pt{h}")
                        nc.scalar.activation(
                            pt[h][:], sc[:], mybir.ActivationFunctionType.Exp,
                            scale=ISD,
                        )
                    for h in range(2):
                        nc.tensor.matmul(
                            ctx[h][:],
                            lhsT=v_sb[:, b * KT + t, h, :],
                            rhs=pt[h][:],
                            start=(t == 0),
                            stop=(t == KT - 1),
                        )
                for h in range(2):
                    hsl = bass.ts(h, D)
                    rec = small.tile([1, SEQT], f32r, tag="rec", name="rec")
                    nc.vector.reciprocal(rec[:], ctx[h][D : D + 1, :])
                    rrep = pp_ps.tile([D, SEQT], f32, tag="pp", name="rrep")
                    nc.tensor.matmul(
                        rrep[:], lhsT=ones64[:], rhs=rec[:], start=True, stop=True
                    )
                    ctmp = small.tile([D, SEQT], f32, tag="ctmp", name="ctmp")
                    nc.vector.tensor_copy(ctmp[:], ctx[h][0:D, :])
                    nc.vector.tensor_tensor(
                        out=ctxT_sb[hsl, qsl],
                        in0=ctmp[:],
                        in1=rrep[:],
                        op=mybir.AluOpType.mult,
                    )

            def outproj_step(m):
                ob = outp.tile([128, E], f32, tag="ob", name="ob")
                for n in range(E // SEQT):
                    ps = pp_ps.tile([128, SEQT], f32, tag="pp", name="ops")
                    nc.tensor.matmul(
                        ps[:],
                        lhsT=ctxT_sb[:, bass.ts(m, 128)],
                        rhs=wo_sb[:, bass.ts(n, SEQT)],
                        start=True,
                        stop=True,
                    )
                    nc.vector.tensor_copy(ob[:, bass.ts(n, SEQT)], ps[:])
                nc.sync.dma_start(out[bass.ts(m, 128), :], ob[:])

            # ---- emission: overlap batches ----
            for st in range(4):           # batch-0 projections
                proj_step(st)
            for qb in range(QB):          # b0 attention // b1 projections
                attn_step(0, qb)
                proj_step(4 + qb)
            for qb in range(QB):          # b1 attention // b0 out-proj
                attn_step(1, qb)
                for m in range(4 * qb, 4 * qb + 4):
                    outproj_step(m)
            for m in range(16, 32):       # b1 out-proj
                outproj_step(m)

    return nc


def _get_program():
    global _PROGRAM
    if _PROGRAM is None:
        _PROGRAM = _build_program()
    return _PROGRAM


def kernel(query, key, value, Wq, bq, Wk, bk, Wv, bv, Wo, bo):
    from concourse.bass_utils import run_bass_kernel_spmd

    nc = _get_program()
    if not getattr(nc, "_waits_split", False):
        _split_excess_waits(nc)
        nc._waits_split = True

    bf = ml_dtypes.bfloat16
    q2 = np.asarray(query, np.float32).reshape(NSEQ, E)
    k2 = np.asarray(key, np.float32).reshape(NSEQ, E)
    v2 = np.asarray(value, np.float32).reshape(NSEQ, E)
    # x^T [E, NSEQ] -> [KE, 128, NSEQ], rounded to bf16 on host (the bf16
    # matmul rounds its inputs anyway)
    xq = np.ascontiguousarray(q2.T).astype(bf).reshape(KE, 128, NSEQ)
    xk = np.ascontiguousarray(k2.T).astype(bf).reshape(KE, 128, NSEQ)
    xv = np.ascontiguousarray(v2.T).astype(bf).reshape(KE, 128, NSEQ)

    Wq = np.asarray(Wq, np.float32)
    Wk = np.asarray(Wk, np.float32)
    Wv = np.asarray(Wv, np.float32)
    Wo = np.asarray(Wo, np.float32)

    in_maps = []
    for c in range(NCORES):
        rsl = slice(DOUT * c, DOUT * (c + 1))
        in_maps.append(
            {
                "xq": xq, "xk": xk, "xv": xv,
                # lhsT for the projections: (W_c)^T [E, DOUT] -> [KE,128,DOUT]
                "wq": np.ascontiguousarray(Wq[rsl, :].T).astype(bf).reshape(KE, 128, DOUT),
                "wk": np.ascontiguousarray(Wk[rsl, :].T).astype(bf).reshape(KE, 128, DOUT),
                "wv": np.ascontiguousarray(Wv[rsl, :].T).astype(bf).reshape(KE, 128, DOUT),
                # rhs for the out-proj: rows c-range of Wo^T  [DOUT, E]
                "wo": np.ascontiguousarray(Wo[:, rsl].T),
                "bq": np.ascontiguousarray(np.asarray(bq, np.float32)[rsl]).reshape(DOUT, 1),
                "bk": np.ascontiguousarray(np.asarray(bk, np.float32)[rsl]).reshape(DOUT, 1),
                "bv": np.ascontiguousarray(np.asarray(bv, np.float32)[rsl]).reshape(DOUT, 1),
            }
        )

    res = run_bass_kernel_spmd(nc, in_maps, list(range(NCORES)), trace=False)
    acc = np.zeros((NSEQ, E), np.float32)
    for c in range(NCORES):
        acc += res.results[c]["out"]
    acc += np.asarray(bo, np.float32)[None, :]
    return acc.reshape(B, S, E)



# revision 13
# speedup vs baseline: 1.2852x; 1.2852x over previous
"""Multihead attention (B=2, S=2048, E=1024, H=16) on 8 TRN2 cores.

Sharding: 2 batch-groups x 4-way head split.  Core c handles batch c//4
and heads {4g..4g+3} where g = c%4 (DOUT = 256 = 2 head-pairs of 128
partition dims).  Each core computes its partial out-projection in bf16;
the host sums the 4 partials per batch and adds the output bias.

Layout / schedule (per core):
  x^T [E, S] streams in bf16 and persists in SBUF.  K-proj and Q-proj
  produce K^T/Q^T [128 = 2 heads x 64, S] per head-pair (contract E on
  partitions, bias via DVE tensor_scalar_add on the PSUM->SBUF move).
  V-proj runs in the transposed orientation (lhsT = x^T chunk, rhs =
  Wv^T) producing V[s, d] chunks directly -- no PE transposes -- with the
  bias folded in as a rank-1 ones x bv matmul; a DVE copy drops them
  into the [V | ones] ctx lhsT ([128 kpos, 65] per head, ones column
  gives the softmax denominator as ctx row 64).
  Attention per (q-block, head-pair): scores for both heads of the pair
  land in one 2-bank PSUM tile [128, 1024]; ONE Exp activation (scale =
  1/sqrt(D)) covers both, halving ACT instruction overhead -- ACT is the
  pacing engine.  ctx accumulates over the 16 kpos tiles in PSUM.
  Normalization: reciprocal_approx_fast on the denominator row (DVE,
  ~5x faster than the exact reciprocal), PE-broadcast via ones64 lhsT
  (f32r bitcast), one DVE multiply into ctxT (bf16).
  Out-proj (contract the 256 local dims, 2 accumulating matmuls) and
  next q-block's Q-proj are interleaved into the attention t-loop so the
  PE never sits behind a phase barrier.
"""

import numpy as np
import ml_dtypes

# Problem constants (hardcoded per the task contract).
B, S, E, H = 2, 2048, 1024, 16
D = E // H            # 64
NCORES = 8
BGROUPS = 2           # batch groups
HSPLIT = NCORES // BGROUPS   # 4-way head split
DOUT = E // HSPLIT    # 256 local head dims = 4 heads = 2 head-pairs
NHP = DOUT // 128     # 2 head-pairs
KE = E // 128         # 8 contraction tiles over E
SEQT = 512            # seq tile for q/k projections and attention q-blocks
NST = S // SEQT       # 4
QB = S // SEQT        # 4 q-blocks
KT = S // 128         # 16 kpos tiles
SC = S // 128         # 16 seq chunks for the V projection
ISD = float(D) ** -0.5

_PROGRAM = None


# ---------------------------------------------------------------------------
# Workarounds for this walrus build: at most ONE sync wait per instruction is
# reliably accepted ("Too many sync wait commands").  (1) tile's final drain
# gets one wait per logical proc — split them over single-wait SP NOPs;
# (2) a general post-pass moves any instruction's excess waits onto
# preceding same-engine NOPs (engine program order preserves semantics).
# ---------------------------------------------------------------------------


def _install_tile_drain_patch():
    import concourse.mybir as mybir
    import concourse.tile as tile
    from concourse.tile import ScopedClock

    if getattr(tile.TileContext, "_drain_patch_installed", False):
        return

    def _patched_drain_and_barrier(self, tick_clock, wait_clock):
        nc = self.nc
        carrier = nc.sync.nop(nofuse=True)
        wait_clock.add_sem_waits(
            carrier.ins, ScopedClock({None: tick_clock.global_clock})
        )
        si = carrier.ins.sync_info
        waits = list(si.on_wait) if si and si.on_wait else []
        ups = list(si.on_update) if si and si.on_update else []
        if len(waits) > 1:
            carrier.ins.sync_info = mybir.SyncInfo(on_wait=[waits[0]], on_update=ups)
            for w in waits[1:]:
                n2 = nc.sync.nop(nofuse=True)
                n2.ins.sync_info = mybir.SyncInfo(on_wait=[w], on_update=[])
        nc.sync.drain()
        nc.all_engine_barrier()
        popped = nc._tile_sem_poison_stack.pop()
        assert popped is self._sem_poison
        nc.clear_and_free_semaphores(list(self.sems.allocated().values()))
        nc.all_engine_barrier()

    tile.TileContext._drain_and_barrier = _patched_drain_and_barrier
    tile.TileContext._drain_patch_installed = True


MAX_WAITS = 1


def _split_excess_waits(nc):
    import concourse.mybir as mybir

    for bb in nc.main_func.blocks:
        il = list(bb.instructions)
        out = []
        changed = False
        for ins in il:
            si = ins.sync_info
            waits = list(si.on_wait) if si and si.on_wait else []
            if len(waits) > MAX_WAITS:
                changed = True
                extras = waits[: len(waits) - MAX_WAITS]
                keep = waits[len(extras):]
                for i in range(0, len(extras), MAX_WAITS):
                    chunk = extras[i : i + MAX_WAITS]
                    nop = mybir.InstNoOp(
                        name=nc.get_next_instruction_name(), ins=[], outs=[]
                    )
                    nop.engine = ins.engine
                    nop.sync_info = mybir.SyncInfo(on_wait=chunk, on_update=[])
                    out.append(nop)
                ins.sync_info = mybir.SyncInfo(
                    on_wait=keep, on_update=list(si.on_update) if si.on_update else []
                )
            out.append(ins)
        if changed:
            bb.instructions = out


def _build_program():
    import concourse.bass as bass
    import concourse.mybir as mybir
    import concourse.tile as tile

    _install_tile_drain_patch()

    f32 = mybir.dt.float32
    f32r = mybir.dt.float32r
    bf16 = mybir.dt.bfloat16
    Exp = mybir.ActivationFunctionType.Exp

    nc = bass.Bass("TRN2", target_bir_lowering=False, debug=False)

    # DRAM I/O (per core).
    xq = nc.dram_tensor("xq", [KE, 128, S], bf16, kind="ExternalInput").ap()
    xk = nc.dram_tensor("xk", [KE, 128, S], bf16, kind="ExternalInput").ap()
    xv = nc.dram_tensor("xv", [KE, 128, S], bf16, kind="ExternalInput").ap()
    wq = nc.dram_tensor("wq", [KE, 128, DOUT], bf16, kind="ExternalInput").ap()
    wk = nc.dram_tensor("wk", [KE, 128, DOUT], bf16, kind="ExternalInput").ap()
    wv = nc.dram_tensor("wv", [KE, 128, DOUT], bf16, kind="ExternalInput").ap()
    wo = nc.dram_tensor("wo", [NHP, 128, E], bf16, kind="ExternalInput").ap()
    bq = nc.dram_tensor("bq", [128, NHP], f32, kind="ExternalInput").ap()
    bk = nc.dram_tensor("bk", [128, NHP], f32, kind="ExternalInput").ap()
    bv = nc.dram_tensor("bv", [1, DOUT], bf16, kind="ExternalInput").ap()
    out = nc.dram_tensor("out", [S, E], bf16, kind="ExternalOutput").ap()

    with tile.TileContext(nc) as tc:
        with (
            nc.allow_low_precision(reason="bf16 attention pipeline"),
            tc.tile_pool(name="consts", bufs=1) as consts,
            tc.tile_pool(name="persist", bufs=1) as persist,
            tc.tile_pool(name="ptp", bufs=3) as ptp,
            tc.tile_pool(name="outp", bufs=4) as outp,
            tc.tile_pool(name="small", bufs=4) as small,
            tc.tile_pool(name="pp_ps", bufs=2, space="PSUM") as pp_ps,
            tc.tile_pool(name="sc_ps", bufs=2, space="PSUM") as sc_ps,
            tc.tile_pool(name="cx_ps", bufs=2, space="PSUM") as cx_ps,
        ):
            # ---- constants ----
            onesf = consts.tile([128, 1], f32)
            nc.vector.memset(onesf[:], 1.0)
            # ones rows (at base partitions 0 and 32) for the reciprocal
            # broadcast matmuls; f32r via copy (memset can't write f32r)
            onesr33 = consts.tile([33, 64], f32r)
            nc.vector.tensor_copy(
                onesr33[:], onesf[0:33, 0:1].broadcast_to([33, 64])
            )
            ones1b = consts.tile([1, 128], bf16)
            nc.vector.memset(ones1b[:], 1.0)

            # ---- persistent weights ----
            wq_sb = persist.tile([128, KE, DOUT], bf16, tag="wq")
            wk_sb = persist.tile([128, KE, DOUT], bf16, tag="wk")
            wv_sb = persist.tile([128, KE, DOUT], bf16, tag="wv")
            wo_sb = persist.tile([128, NHP, E], bf16, tag="wo")
            bq_sb = persist.tile([128, NHP], f32, tag="bq")
            bk_sb = persist.tile([128, NHP], f32, tag="bk")
            bv_sb = persist.tile([1, DOUT], bf16, tag="bv")

            # ---- persistent activations ----
            xq_sb = persist.tile([128, KE, S], bf16, tag="xq")
            xk_sb = persist.tile([128, KE, S], bf16, tag="xk")
            xv_sb = persist.tile([128, KE, S], bf16, tag="xv")
            qt_sb = persist.tile([128, NHP, S], bf16, tag="qt")
            kt_sb = persist.tile([128, NHP, S], bf16, tag="kt")
            # [V | ones] per (kpos chunk, local head): ctx lhsT
            v_sb = persist.tile([128, SC, 4, D + 1], bf16, tag="vn")
            ctxT_sb = persist.tile([128, NHP, S], bf16, tag="ctxT")

            nc.vector.tensor_copy(
                v_sb[:, :, :, D], onesf[:, 0:1].broadcast_to([128, SC, 4])
            )

            # ---- input DMAs, K/V/Q order so the PE can start ASAP ----
            for k in range(KE):
                nc.sync.dma_start(wk_sb[:, k, :], wk[k])
            nc.sync.dma_start(bk_sb[:], bk[:])
            nc.sync.dma_start(bq_sb[:], bq[:])
            for k in range(KE):
                nc.sync.dma_start(xk_sb[:, k, :], xk[k])
            for k in range(KE):
                nc.sync.dma_start(wv_sb[:, k, :], wv[k])
            nc.sync.dma_start(bv_sb[:], bv[:])
            for k in range(KE):
                nc.sync.dma_start(xv_sb[:, k, :], xv[k])
            for k in range(KE):
                nc.sync.dma_start(wq_sb[:, k, :], wq[k])
            for k in range(KE):
                nc.sync.dma_start(xq_sb[:, k, :], xq[k])
            for hp in range(NHP):
                nc.sync.dma_start(wo_sb[:, hp, :], wo[hp])

            def kq_proj(name, w_sb, x_sb, b_sb, dst, st):
                """One [128, 512] tile of the K or Q projection (transposed
                output layout [d, s]) for head-pair slice + bias add."""
                sl = bass.ts(st, SEQT)
                for hp in range(NHP):
                    ps = pp_ps.tile([128, SEQT], f32, tag="pp", name=f"{name}{st}{hp}")
                    for k in range(KE):
                        nc.tensor.matmul(
                            ps[:],
                            lhsT=w_sb[:, k, bass.ts(hp, 128)],
                            rhs=x_sb[:, k, sl],
                            start=(k == 0),
                            stop=(k == KE - 1),
                        )
                    nc.vector.tensor_scalar_add(
                        dst[:, hp, sl], ps[:], b_sb[:, hp : hp + 1]
                    )

            def v_proj(sc):
                """V rows [128 seq, 256 d] directly via lhsT = x^T chunk;
                bias folded in as a rank-1 (ones x bv) accumulate."""
                ssl = bass.ts(sc, 128)
                ps = pp_ps.tile([128, SEQT], f32, tag="pp", name=f"vp{sc}")
                vp = ps[:, 0:DOUT]
                for k in range(KE):
                    nc.tensor.matmul(
                        vp,
                        lhsT=xv_sb[:, k, ssl],
                        rhs=wv_sb[:, k, :],
                        start=(k == 0),
                        stop=False,
                    )
                nc.tensor.matmul(
                    vp, lhsT=ones1b[:], rhs=bv_sb[:], start=False, stop=True
                )
                for h in range(4):
                    nc.vector.tensor_copy(
                        v_sb[:, sc, h, 0:D], ps[:, bass.ts(h, D)]
                    )

            def outproj_m(qb, m):
                """One output row-tile [128 seq, E]: contract the 256 local
                dims (2 accumulating matmuls per 512-wide chunk), copy to
                SBUF bf16, DMA out."""
                msl = bass.ts(4 * qb + m, 128)
                for n in range(E // SEQT):
                    ps = pp_ps.tile([128, SEQT], f32, tag="pp", name=f"ob{qb}{m}{n}")
                    for hp in range(NHP):
                        nc.tensor.matmul(
                            ps[:],
                            lhsT=ctxT_sb[:, hp, msl],
                            rhs=wo_sb[:, hp, bass.ts(n, SEQT)],
                            start=(hp == 0),
                            stop=(hp == NHP - 1),
                        )
                    ob = outp.tile([128, SEQT], bf16, tag="ob", name="ob")
                    nc.vector.tensor_copy(ob[:], ps[:])
                    nc.sync.dma_start(out[msl, bass.ts(n, SEQT)], ob[:])

            def attention(qb, side):
                """Attention for q-block qb (both head-pairs), with `side`
                closures (out-proj / next Q-proj) sprinkled into the t-loop
                to keep the PE busy while ACT paces the exps.

                Normalization: each head's unnormalized ctx rows are copied
                to SBUF (bf16) and its denominator row to partition 32*lh of
                a shared tile; ONE exact DVE reciprocal per q-block covers
                all 4 heads, then a per-head PE broadcast (ones row at the
                matching base partition) and a DVE multiply write ctxT."""
                qsl = bass.ts(qb, SEQT)
                for hp in range(NHP):
                    ctx0 = cx_ps.tile([D + 1, SEQT], f32, tag="cx", name="ctx0")
                    ctx1 = cx_ps.tile([D + 1, SEQT], f32, tag="cx", name="ctx1")
                    ctx = (ctx0, ctx1)
                    for t in range(KT):
                        ksl = bass.ts(t, 128)
                        sc2 = sc_ps.tile([128, 2 * SEQT], f32, tag="sc", name="sc2")
                        for h in range(2):
                            hsl = bass.ts(h, D)
                            nc.tensor.matmul(
                                sc2[:, bass.ts(h, SEQT)],
                                lhsT=kt_sb[hsl, hp, ksl],
                                rhs=qt_sb[hsl, hp, qsl],
                                start=True,
                                stop=True,
                            )
                        pt = ptp.tile([128, 2 * SEQT], bf16, tag="pt", name="pt")
                        nc.scalar.activation(pt[:], sc2[:], Exp, scale=ISD)
                        for h in range(2):
                            nc.tensor.matmul(
                                ctx[h][:],
                                lhsT=v_sb[:, t, 2 * hp + h, :],
                                rhs=pt[:, bass.ts(h, SEQT)],
                                start=(t == 0),
                                stop=(t == KT - 1),
                            )
                        if side:
                            side.pop(0)()
                    dn = small.tile([33, SEQT], f32, tag="dn", name="dn")
                    ctmp = [None, None]
                    for h in range(2):
                        nc.vector.tensor_copy(
                            dn[32 * h : 32 * h + 1, :], ctx[h][D : D + 1, :]
                        )
                        ctmp[h] = small.tile(
                            [D, SEQT], bf16, tag=f"ctmp{h}", name="ctmp"
                        )
                        nc.vector.tensor_copy(ctmp[h][:], ctx[h][0:D, :])
                    recf = small.tile([33, SEQT], f32, tag="recf", name="recf")
                    nc.vector.reciprocal(recf[:], dn[:])
                    recr = small.tile([33, SEQT], f32r, tag="recr", name="recr")
                    nc.vector.tensor_copy(recr[:], recf[:])
                    for h in range(2):
                        p = 32 * h
                        rrep = pp_ps.tile([D, SEQT], f32, tag="pp", name="rrep")
                        nc.tensor.matmul(
                            rrep[:],
                            lhsT=onesr33[p : p + 1, :],
                            rhs=recr[p : p + 1, :],
                            start=True,
                            stop=True,
                        )
                        nc.vector.tensor_tensor(
                            out=ctxT_sb[bass.ds(h * D, D), hp, qsl],
                            in0=ctmp[h][:],
                            in1=rrep[:],
                            op=mybir.AluOpType.mult,
                        )
                while side:
                    side.pop(0)()

            # ---- emission ----
            for st in range(NST):
                kq_proj("kp", wk_sb, xk_sb, bk_sb, kt_sb, st)
            for sc in range(SC):
                v_proj(sc)
            kq_proj("qp", wq_sb, xq_sb, bq_sb, qt_sb, 0)

            for qb in range(QB):
                side = []
                if qb + 1 < QB:
                    side.append(
                        lambda st=qb + 1: kq_proj(
                            "qp", wq_sb, xq_sb, bq_sb, qt_sb, st
                        )
                    )
                if qb > 0:
                    for m in range(4):
                        side.append(lambda q=qb - 1, m=m: outproj_m(q, m))
                attention(qb, side)
            for m in range(4):
                outproj_m(QB - 1, m)

    return nc


def _get_program():
    global _PROGRAM
    if _PROGRAM is None:
        _PROGRAM = _build_program()
    return _PROGRAM


def kernel(query, key, value, Wq, bq, Wk, bk, Wv, bv, Wo, bo):
    from concourse.bass_utils import run_bass_kernel_spmd

    nc = _get_program()
    if not getattr(nc, "_waits_split", False):
        _split_excess_waits(nc)
        nc._waits_split = True

    bf = ml_dtypes.bfloat16
    query = np.asarray(query, np.float32)
    key = np.asarray(key, np.float32)
    value = np.asarray(value, np.float32)
    Wq = np.asarray(Wq, np.float32)
    Wk = np.asarray(Wk, np.float32)
    Wv = np.asarray(Wv, np.float32)
    Wo = np.asarray(Wo, np.float32)
    bq = np.asarray(bq, np.float32)
    bk = np.asarray(bk, np.float32)
    bv = np.asarray(bv, np.float32)
    bo = np.asarray(bo, np.float32)

    # Per-batch x^T [E, S] -> [KE, 128, S] bf16
    xT = {}
    for b in range(B):
        xT[("q", b)] = np.ascontiguousarray(query[b].T).astype(bf).reshape(KE, 128, S)
        xT[("k", b)] = np.ascontiguousarray(key[b].T).astype(bf).reshape(KE, 128, S)
        xT[("v", b)] = np.ascontiguousarray(value[b].T).astype(bf).reshape(KE, 128, S)

    in_maps = []
    for c in range(NCORES):
        b = c // HSPLIT
        g = c % HSPLIT
        rsl = slice(DOUT * g, DOUT * (g + 1))
        in_maps.append(
            {
                "xq": xT[("q", b)], "xk": xT[("k", b)], "xv": xT[("v", b)],
                # lhsT for q/k (and rhs for v): (W_g)^T [E, DOUT]
                "wq": np.ascontiguousarray(Wq[rsl, :].T).astype(bf).reshape(KE, 128, DOUT),
                "wk": np.ascontiguousarray(Wk[rsl, :].T).astype(bf).reshape(KE, 128, DOUT),
                "wv": np.ascontiguousarray(Wv[rsl, :].T).astype(bf).reshape(KE, 128, DOUT),
                # rhs for the out-proj: rows g-range of Wo^T as [NHP, 128, E]
                "wo": np.ascontiguousarray(Wo[:, rsl].T).astype(bf).reshape(NHP, 128, E),
                "bq": np.ascontiguousarray(bq[rsl].reshape(NHP, 128).T),
                "bk": np.ascontiguousarray(bk[rsl].reshape(NHP, 128).T),
                "bv": np.ascontiguousarray(bv[rsl].reshape(1, DOUT)).astype(bf),
            }
        )

    res = run_bass_kernel_spmd(nc, in_maps, list(range(NCORES)), trace=False)
    full = np.empty((B, S, E), np.float32)
    for b in range(B):
        acc = np.zeros((S, E), np.float32)
        for g in range(HSPLIT):
            acc += np.asarray(res.results[b * HSPLIT + g]["out"], np.float32)
        full[b] = acc + bo[None, :]
    return full


# revision 18
# speedup vs baseline: 1.5709x; 1.2223x over previous
"""Multihead attention (B=2, S=2048, E=1024, H=16) on 8 TRN2 cores.

Sharding: 2 batch-groups x 4-way head split.  Core c handles batch c//4
and heads {4g..4g+3} where g = c%4 (DOUT = 256 = 2 head-pairs of 128
partition dims).  Each core computes its partial out-projection in bf16;
the host sums the 4 partials per batch and adds the output bias.

Layout / schedule (per core):
  x^T [E, S] streams in bf16 and persists in SBUF.  K-proj and Q-proj
  produce K^T/Q^T [128 = 2 heads x 64, S] per head-pair (contract E on
  partitions, bias via DVE tensor_scalar_add on the PSUM->SBUF move).
  V-proj runs in the transposed orientation (lhsT = x^T chunk, rhs =
  Wv^T) producing V[s, d] chunks directly -- no PE transposes -- with the
  bias folded in as a rank-1 ones x bv matmul; a DVE copy drops them
  into the [V | ones] ctx lhsT ([128 kpos, 65] per head, ones column
  gives the softmax denominator as ctx row 64).
  Attention per (q-block, head-pair): scores for both heads of the pair
  land in one 2-bank PSUM tile [128, 1024]; ONE Exp activation (scale =
  1/sqrt(D)) covers both, halving ACT instruction overhead -- ACT is the
  pacing engine.  ctx accumulates over the 16 kpos tiles in PSUM.
  Normalization: reciprocal_approx_fast on the denominator row (DVE,
  ~5x faster than the exact reciprocal), PE-broadcast via ones64 lhsT
  (f32r bitcast), one DVE multiply into ctxT (bf16).
  Out-proj (contract the 256 local dims, 2 accumulating matmuls) and
  next q-block's Q-proj are interleaved into the attention t-loop so the
  PE never sits behind a phase barrier.
"""

import numpy as np
import ml_dtypes

# Problem constants (hardcoded per the task contract).
B, S, E, H = 2, 2048, 1024, 16
D = E // H            # 64
NCORES = 8
BGROUPS = 2           # batch groups
HSPLIT = NCORES // BGROUPS   # 4-way head split
DOUT = E // HSPLIT    # 256 local head dims = 4 heads = 2 head-pairs
NHP = DOUT // 128     # 2 head-pairs
KE = E // 128         # 8 contraction tiles over E
SEQT = 512            # seq tile for q/k projections and attention q-blocks
NST = S // SEQT       # 4
QB = S // SEQT        # 4 q-blocks
KT = S // 128         # 16 kpos tiles
SC = S // 128         # 16 seq chunks for the V projection
ISD = float(D) ** -0.5

_PROGRAM = None


# ---------------------------------------------------------------------------
# Workarounds for this walrus build: at most ONE sync wait per instruction is
# reliably accepted ("Too many sync wait commands").  (1) tile's final drain
# gets one wait per logical proc — split them over single-wait SP NOPs;
# (2) a general post-pass moves any instruction's excess waits onto
# preceding same-engine NOPs (engine program order preserves semantics).
# ---------------------------------------------------------------------------


def _install_tile_drain_patch():
    import concourse.mybir as mybir
    import concourse.tile as tile
    from concourse.tile import ScopedClock

    if getattr(tile.TileContext, "_drain_patch_installed", False):
        return

    def _patched_drain_and_barrier(self, tick_clock, wait_clock):
        nc = self.nc
        carrier = nc.sync.nop(nofuse=True)
        wait_clock.add_sem_waits(
            carrier.ins, ScopedClock({None: tick_clock.global_clock})
        )
        si = carrier.ins.sync_info
        waits = list(si.on_wait) if si and si.on_wait else []
        ups = list(si.on_update) if si and si.on_update else []
        if len(waits) > 1:
            carrier.ins.sync_info = mybir.SyncInfo(on_wait=[waits[0]], on_update=ups)
            for w in waits[1:]:
                n2 = nc.sync.nop(nofuse=True)
                n2.ins.sync_info = mybir.SyncInfo(on_wait=[w], on_update=[])
        nc.sync.drain()
        nc.all_engine_barrier()
        popped = nc._tile_sem_poison_stack.pop()
        assert popped is self._sem_poison
        nc.clear_and_free_semaphores(list(self.sems.allocated().values()))
        nc.all_engine_barrier()

    tile.TileContext._drain_and_barrier = _patched_drain_and_barrier
    tile.TileContext._drain_patch_installed = True


MAX_WAITS = 1


def _split_excess_waits(nc):
    import concourse.mybir as mybir

    for bb in nc.main_func.blocks:
        il = list(bb.instructions)
        out = []
        changed = False
        for ins in il:
            si = ins.sync_info
            waits = list(si.on_wait) if si and si.on_wait else []
            if len(waits) > MAX_WAITS:
                changed = True
                extras = waits[: len(waits) - MAX_WAITS]
                keep = waits[len(extras):]
                for i in range(0, len(extras), MAX_WAITS):
                    chunk = extras[i : i + MAX_WAITS]
                    nop = mybir.InstNoOp(
                        name=nc.get_next_instruction_name(), ins=[], outs=[]
                    )
                    nop.engine = ins.engine
                    nop.sync_info = mybir.SyncInfo(on_wait=chunk, on_update=[])
                    out.append(nop)
                ins.sync_info = mybir.SyncInfo(
                    on_wait=keep, on_update=list(si.on_update) if si.on_update else []
                )
            out.append(ins)
        if changed:
            bb.instructions = out


def _build_program():
    import concourse.bass as bass
    import concourse.mybir as mybir
    import concourse.tile as tile

    _install_tile_drain_patch()

    f32 = mybir.dt.float32
    f32r = mybir.dt.float32r
    bf16 = mybir.dt.bfloat16
    Exp = mybir.ActivationFunctionType.Exp

    nc = bass.Bass("TRN2", target_bir_lowering=False, debug=False)

    # DRAM I/O (per core).
    xq = nc.dram_tensor("xq", [KE, 128, S], bf16, kind="ExternalInput").ap()
    xk = nc.dram_tensor("xk", [KE, 128, S], bf16, kind="ExternalInput").ap()
    xv = nc.dram_tensor("xv", [KE, 128, S], bf16, kind="ExternalInput").ap()
    wq = nc.dram_tensor("wq", [KE, 128, DOUT], bf16, kind="ExternalInput").ap()
    wk = nc.dram_tensor("wk", [KE, 128, DOUT], bf16, kind="ExternalInput").ap()
    wv = nc.dram_tensor("wv", [KE, 128, DOUT], bf16, kind="ExternalInput").ap()
    wo = nc.dram_tensor("wo", [NHP, 128, E], bf16, kind="ExternalInput").ap()
    bq = nc.dram_tensor("bq", [128, NHP], f32, kind="ExternalInput").ap()
    bk = nc.dram_tensor("bk", [128, NHP], f32, kind="ExternalInput").ap()
    bv = nc.dram_tensor("bv", [1, DOUT], bf16, kind="ExternalInput").ap()
    out = nc.dram_tensor("out", [S, E], bf16, kind="ExternalOutput").ap()

    with tile.TileContext(nc) as tc:
        with (
            nc.allow_low_precision(reason="bf16 attention pipeline"),
            tc.tile_pool(name="consts", bufs=1) as consts,
            tc.tile_pool(name="persist", bufs=1) as persist,
            tc.tile_pool(name="ptp", bufs=3) as ptp,
            tc.tile_pool(name="outp", bufs=4) as outp,
            tc.tile_pool(name="small", bufs=4) as small,
            tc.tile_pool(name="pp_ps", bufs=2, space="PSUM") as pp_ps,
            tc.tile_pool(name="sc_ps", bufs=2, space="PSUM") as sc_ps,
            tc.tile_pool(name="cx_ps", bufs=2, space="PSUM") as cx_ps,
        ):
            # ---- constants ----
            onesf = consts.tile([128, 1], f32)
            nc.vector.memset(onesf[:], 1.0)
            # ones rows (at base partitions 0 and 32) for the reciprocal
            # broadcast matmuls; f32r via copy (memset can't write f32r)
            onesr33 = consts.tile([33, 64], f32r)
            nc.vector.tensor_copy(
                onesr33[:], onesf[0:33, 0:1].broadcast_to([33, 64])
            )
            ones1b = consts.tile([1, 128], bf16)
            nc.vector.memset(ones1b[:], 1.0)

            # ---- persistent weights ----
            wq_sb = persist.tile([128, KE, DOUT], bf16, tag="wq")
            wk_sb = persist.tile([128, KE, DOUT], bf16, tag="wk")
            wv_sb = persist.tile([128, KE, DOUT], bf16, tag="wv")
            wo_sb = persist.tile([128, NHP, E], bf16, tag="wo")
            bq_sb = persist.tile([128, NHP], f32, tag="bq")
            bk_sb = persist.tile([128, NHP], f32, tag="bk")
            bv_sb = persist.tile([1, DOUT], bf16, tag="bv")

            # ---- persistent activations ----
            xq_sb = persist.tile([128, KE, S], bf16, tag="xq")
            xk_sb = persist.tile([128, KE, S], bf16, tag="xk")
            xv_sb = persist.tile([128, KE, S], bf16, tag="xv")
            qt_sb = persist.tile([128, NHP, S], bf16, tag="qt")
            kt_sb = persist.tile([128, NHP, S], bf16, tag="kt")
            # [V | ones] per (kpos chunk, local head): ctx lhsT
            v_sb = persist.tile([128, SC, 4, D + 1], bf16, tag="vn")
            ctxT_sb = persist.tile([128, NHP, S], bf16, tag="ctxT")

            nc.vector.tensor_copy(
                v_sb[:, :, :, D], onesf[:, 0:1].broadcast_to([128, SC, 4])
            )

            # ---- input DMAs, ordered by first use and chunked by seq tile
            # so the consuming matmuls start as early as possible ----
            for k in range(KE):
                nc.sync.dma_start(wk_sb[:, k, :], wk[k])
            nc.sync.dma_start(bk_sb[:], bk[:])
            nc.sync.dma_start(bq_sb[:], bq[:])
            for k in range(KE):
                nc.sync.dma_start(wq_sb[:, k, :], wq[k])
            for st in range(NST):
                sl = bass.ts(st, SEQT)
                for k in range(KE):
                    nc.sync.dma_start(xk_sb[:, k, sl], xk[k, :, sl])
                if st == 0:
                    for k in range(KE):
                        nc.sync.dma_start(xq_sb[:, k, sl], xq[k, :, sl])
            for k in range(KE):
                nc.sync.dma_start(wv_sb[:, k, :], wv[k])
            nc.sync.dma_start(bv_sb[:], bv[:])
            for scg in range(NST):
                sl = bass.ts(scg, SEQT)
                for k in range(KE):
                    nc.sync.dma_start(xv_sb[:, k, sl], xv[k, :, sl])
            for st in range(1, NST):
                sl = bass.ts(st, SEQT)
                for k in range(KE):
                    nc.sync.dma_start(xq_sb[:, k, sl], xq[k, :, sl])
            for hp in range(NHP):
                nc.sync.dma_start(wo_sb[:, hp, :], wo[hp])

            def kq_proj_hp(name, w_sb, x_sb, b_sb, dst, st, hp):
                """One [128, 512] tile of the K or Q projection (transposed
                output layout [d, s]) for one head-pair + bias add."""
                sl = bass.ts(st, SEQT)
                ps = pp_ps.tile([128, SEQT], f32, tag="pp", name=f"{name}{st}{hp}")
                for k in range(KE):
                    nc.tensor.matmul(
                        ps[:],
                        lhsT=w_sb[:, k, bass.ts(hp, 128)],
                        rhs=x_sb[:, k, sl],
                        start=(k == 0),
                        stop=(k == KE - 1),
                    )
                nc.vector.tensor_scalar_add(
                    dst[:, hp, sl], ps[:], b_sb[:, hp : hp + 1]
                )

            def kq_proj(name, w_sb, x_sb, b_sb, dst, st):
                for hp in range(NHP):
                    kq_proj_hp(name, w_sb, x_sb, b_sb, dst, st, hp)

            def v_proj(sc):
                """V rows [128 seq, 256 d] directly via lhsT = x^T chunk;
                bias folded in as a rank-1 (ones x bv) accumulate."""
                ssl = bass.ts(sc, 128)
                ps = pp_ps.tile([128, SEQT], f32, tag="pp", name=f"vp{sc}")
                vp = ps[:, 0:DOUT]
                for k in range(KE):
                    nc.tensor.matmul(
                        vp,
                        lhsT=xv_sb[:, k, ssl],
                        rhs=wv_sb[:, k, :],
                        start=(k == 0),
                        stop=False,
                    )
                nc.tensor.matmul(
                    vp, lhsT=ones1b[:], rhs=bv_sb[:], start=False, stop=True
                )
                for h in range(4):
                    nc.vector.tensor_copy(
                        v_sb[:, sc, h, 0:D], ps[:, bass.ts(h, D)]
                    )

            def outproj_m(qb, m):
                """One output row-tile [128 seq, E]: contract the 256 local
                dims (2 accumulating matmuls per 512-wide chunk), copy to
                SBUF bf16, DMA out."""
                msl = bass.ts(4 * qb + m, 128)
                for n in range(E // SEQT):
                    ps = pp_ps.tile([128, SEQT], f32, tag="pp", name=f"ob{qb}{m}{n}")
                    for hp in range(NHP):
                        nc.tensor.matmul(
                            ps[:],
                            lhsT=ctxT_sb[:, hp, msl],
                            rhs=wo_sb[:, hp, bass.ts(n, SEQT)],
                            start=(hp == 0),
                            stop=(hp == NHP - 1),
                        )
                    ob = outp.tile([128, SEQT], bf16, tag="ob", name="ob")
                    nc.vector.tensor_copy(ob[:], ps[:])
                    nc.sync.dma_start(out[msl, bass.ts(n, SEQT)], ob[:])

            def norm_part1(qb, hp, ctx):
                """DVE-only half of the softmax normalization: pull the
                denominator rows + unnormalized ctx out of PSUM (freeing the
                cx banks) and compute the reciprocals.  No PE instructions,
                so the next head-pair's scores are not blocked behind the
                serial reciprocal chain."""
                dn = small.tile([33, SEQT], f32, tag="dn", name="dn", bufs=2)
                ctmp = [None, None]
                for h in range(2):
                    nc.vector.tensor_copy(
                        dn[32 * h : 32 * h + 1, :], ctx[h][D : D + 1, :]
                    )
                    ctmp[h] = small.tile(
                        [D, SEQT], bf16, tag=f"ctmp{hp}{h}", name="ctmp",
                        bufs=2,
                    )
                    nc.vector.tensor_copy(ctmp[h][:], ctx[h][0:D, :])
                recf = small.tile([33, SEQT], f32, tag=f"recf{hp}", name="recf", bufs=2)
                nc.vector.reciprocal(recf[:], dn[:])
                recr = small.tile([33, SEQT], f32r, tag=f"recr{hp}", name="recr", bufs=2)
                nc.vector.tensor_copy(recr[:], recf[:])
                return ctmp, recr

            def norm_part2(qb, hp, ctmp, recr):
                """PE broadcast of the reciprocals + DVE multiply into ctxT.
                Emitted as a deferred slot closure."""
                qsl = bass.ts(qb, SEQT)
                for h in range(2):
                    p = 32 * h
                    rrep = pp_ps.tile([D, SEQT], f32, tag="pp", name="rrep")
                    nc.tensor.matmul(
                        rrep[:],
                        lhsT=onesr33[p : p + 1, :],
                        rhs=recr[p : p + 1, :],
                        start=True,
                        stop=True,
                    )
                    nc.vector.tensor_tensor(
                        out=ctxT_sb[bass.ds(h * D, D), hp, qsl],
                        in0=ctmp[h][:],
                        in1=rrep[:],
                        op=mybir.AluOpType.mult,
                    )

            def attention(qb, slots):
                """Attention for q-block qb (both head-pairs).  `slots` maps
                slot index (hp*KT + t) -> list of closures emitted between
                that step's exp and ctx matmuls, keeping the PE busy while
                ACT paces the exps.  Returns the norm state for the deferred
                norm_part2."""
                qsl = bass.ts(qb, SEQT)
                norm_state = []
                for hp in range(NHP):
                    ctx0 = cx_ps.tile([D + 1, SEQT], f32, tag="cx", name="ctx0")
                    ctx1 = cx_ps.tile([D + 1, SEQT], f32, tag="cx", name="ctx1")
                    ctx = (ctx0, ctx1)
                    for t in range(KT):
                        ksl = bass.ts(t, 128)
                        sc2 = sc_ps.tile([128, 2 * SEQT], f32, tag="sc", name="sc2")
                        for h in range(2):
                            hsl = bass.ts(h, D)
                            nc.tensor.matmul(
                                sc2[:, bass.ts(h, SEQT)],
                                lhsT=kt_sb[hsl, hp, ksl],
                                rhs=qt_sb[hsl, hp, qsl],
                                start=True,
                                stop=True,
                            )
                        pt = ptp.tile([128, 2 * SEQT], bf16, tag="pt", name="pt")
                        nc.scalar.activation(pt[:], sc2[:], Exp, scale=ISD)
                        for fn in slots.get(hp * KT + t, ()):
                            fn()
                        for h in range(2):
                            nc.tensor.matmul(
                                ctx[h][:],
                                lhsT=v_sb[:, t, 2 * hp + h, :],
                                rhs=pt[:, bass.ts(h, SEQT)],
                                start=(t == 0),
                                stop=(t == KT - 1),
                            )
                    norm_state.append((qb, hp) + tuple(norm_part1(qb, hp, ctx)))
                return norm_state

            # ---- emission ----
            # Upfront (DMA-paced): K-proj st0/st1 + Q-proj(qb0); everything
            # else rides inside the attention slot schedule.
            kq_proj("kp", wk_sb, xk_sb, bk_sb, kt_sb, 0)
            kq_proj("kp", wk_sb, xk_sb, bk_sb, kt_sb, 1)
            kq_proj("qp", wq_sb, xq_sb, bq_sb, qt_sb, 0)

            def add(slots, i, fn):
                slots.setdefault(i, []).append(fn)

            pending_norm = []
            pending_out = []
            for qb in range(QB):
                slots = {}
                if qb == 0:
                    # V-proj(t) lands right before ctx(t) needs it; the
                    # remaining K-proj tiles arrive before scores reach them.
                    for t in range(SC):
                        add(slots, t, lambda sc=t: v_proj(sc))
                    for hp in range(NHP):
                        add(slots, 2 + 2 * hp, lambda hp=hp: kq_proj_hp(
                            "kp", wk_sb, xk_sb, bk_sb, kt_sb, 2, hp))
                        add(slots, 8 + 2 * hp, lambda hp=hp: kq_proj_hp(
                            "kp", wk_sb, xk_sb, bk_sb, kt_sb, 3, hp))
                    for hp in range(NHP):
                        add(slots, KT + 1 + 2 * hp, lambda hp=hp: kq_proj_hp(
                            "qp", wq_sb, xq_sb, bq_sb, qt_sb, 1, hp))
                else:
                    for i, ns in enumerate(pending_norm):
                        add(slots, 1 + i, lambda ns=ns: norm_part2(*ns))
                    if qb + 1 < QB:
                        for hp in range(NHP):
                            add(slots, 3 + 2 * hp, lambda st=qb + 1, hp=hp:
                                kq_proj_hp("qp", wq_sb, xq_sb, bq_sb,
                                           qt_sb, st, hp))
                    for m, (q, mm) in enumerate(pending_out):
                        add(slots, 8 + 4 * m, lambda q=q, mm=mm: outproj_m(q, mm))
                pending_norm = attention(qb, slots)
                pending_out = [(qb, m) for m in range(4)]
            for ns in pending_norm:
                norm_part2(*ns)
            for q, m in pending_out:
                outproj_m(q, m)

    return nc


def _get_program():
    global _PROGRAM
    if _PROGRAM is None:
        _PROGRAM = _build_program()
    return _PROGRAM


def kernel(query, key, value, Wq, bq, Wk, bk, Wv, bv, Wo, bo):
    from concourse.bass_utils import run_bass_kernel_spmd

    nc = _get_program()
    if not getattr(nc, "_waits_split", False):
        _split_excess_waits(nc)
        nc._waits_split = True

    bf = ml_dtypes.bfloat16
    query = np.asarray(query, np.float32)
    key = np.asarray(key, np.float32)
    value = np.asarray(value, np.float32)
    Wq = np.asarray(Wq, np.float32)
    Wk = np.asarray(Wk, np.float32)
    Wv = np.asarray(Wv, np.float32)
    Wo = np.asarray(Wo, np.float32)
    bq = np.asarray(bq, np.float32)
    bk = np.asarray(bk, np.float32)
    bv = np.asarray(bv, np.float32)
    bo = np.asarray(bo, np.float32)

    # Per-batch x^T [E, S] -> [KE, 128, S] bf16
    xT = {}
    for b in range(B):
        xT[("q", b)] = np.ascontiguousarray(query[b].T).astype(bf).reshape(KE, 128, S)
        xT[("k", b)] = np.ascontiguousarray(key[b].T).astype(bf).reshape(KE, 128, S)
        xT[("v", b)] = np.ascontiguousarray(value[b].T).astype(bf).reshape(KE, 128, S)

    in_maps = []
    for c in range(NCORES):
        b = c // HSPLIT
        g = c % HSPLIT
        rsl = slice(DOUT * g, DOUT * (g + 1))
        in_maps.append(
            {
                "xq": xT[("q", b)], "xk": xT[("k", b)], "xv": xT[("v", b)],
                # lhsT for q/k (and rhs for v): (W_g)^T [E, DOUT]
                "wq": np.ascontiguousarray(Wq[rsl, :].T).astype(bf).reshape(KE, 128, DOUT),
                "wk": np.ascontiguousarray(Wk[rsl, :].T).astype(bf).reshape(KE, 128, DOUT),
                "wv": np.ascontiguousarray(Wv[rsl, :].T).astype(bf).reshape(KE, 128, DOUT),
                # rhs for the out-proj: rows g-range of Wo^T as [NHP, 128, E]
                "wo": np.ascontiguousarray(Wo[:, rsl].T).astype(bf).reshape(NHP, 128, E),
                "bq": np.ascontiguousarray(bq[rsl].reshape(NHP, 128).T),
                "bk": np.ascontiguousarray(bk[rsl].reshape(NHP, 128).T),
                "bv": np.ascontiguousarray(bv[rsl].reshape(1, DOUT)).astype(bf),
            }
        )

    res = run_bass_kernel_spmd(nc, in_maps, list(range(NCORES)), trace=False)
    full = np.empty((B, S, E), np.float32)
    for b in range(B):
        acc = np.zeros((S, E), np.float32)
        for g in range(HSPLIT):
            acc += np.asarray(res.results[b * HSPLIT + g]["out"], np.float32)
        full[b] = acc + bo[None, :]
    return full
